# revision 44
# baseline (speedup 1.0000x reference)
"""Depthwise 3x3 conv (SAME, channel multiplier 2) on [16,224,224,96] f32,
data-parallel over batch across 8 TRN2 NeuronCores.

Per-core mapping (2 images/core): the conv along H is expressed as a banded
matmul on TensorE — stationary [116,112] band matrix whose 3 diagonals hold
the kernel column k[:, dw, m]; contract dim = 114 input rows (112 out rows +
halo) + 2 bias rows (all-ones coefficients fold the bias add into the PE).
The 3 W-shifts accumulate into PSUM via the moving operand's column offset
(dw*96 in the flattened (w,c) free dim).  Operands stream as f16 (matmul
cost is N moving columns @2.4GHz regardless of dtype/K; PE is the wall at
~217us/core: 24 passes over the (w,c) plane is a conserved quantity of any
banded formulation — M<=128 PSUM partitions x 3 taps x 2 mult).

PSUM [112, 512] per (chunk, m) pairs into [112,1024] tiles; DVE/ACT
interleave-copy them (out ch = 2c+m -> stride-2 write) into SBUF groups of 7
chunks.  The output is written to HBM as f16 and upcast on the host — the
2e-2 rel-err budget absorbs the ~3e-4 rounding and it halves the dominant
(write) HBM stream: 96 -> 58 MB/core total traffic.

Schedule trims: first tile ladders x/bias/band loads finely across both
HWDGE queues (cold DMA only sustains ~50GB/s for the first ~5us; first
matmul ~12.7us), last tile tapers output groups (7,7,4,2,1), splits the
final two chunks' drains across DVE+ACT and spreads the closing DMAs over
both queues.  Measured 247us/core (335us baseline): ~12.7us ramp + ~225us
PE-bound stream + ~8us tail.  NOTE the device has two clock states —
216ns/MM (2.4GHz) and 259ns/MM (~2.0GHz, thermal): compare runs only at
matching median MM gap.  Measured neutral-or-worse: FWL 128-col pad
(neutral — LDWEIGHTS already hidden), fp8 (fails 2e-2), every-chunk split
drains (+3us), interleaved stride-2 matmul PSUM dst (breaks accumulation),
engine direct-conv offload (engines are partition-locked; conv taps need
cross-partition reads).
"""

import sys

sys.path.insert(0, "/opt/trn_rl_repo")

import numpy as np

B, H, W, C = 16, 224, 224, 96
MULT = 2
NCORES = 8
BPC = B // NCORES  # images per core
M = 112            # output rows per h-tile
MB = 112           # band stationary columns (128-pad triggers FWL: measured neutral —
                   # LDWEIGHTS 131->98ns but already hidden under the 216ns/MM stream)
KP = 116           # contract partitions: 114 x rows + 2 bias rows
WH = 112           # w-half width
COLS = (WH + 2) * C         # 10944 x-tile cols (1-w halo each side)
CHUNK = 512
NCHUNK = WH * C // CHUNK    # 21
GRP = 7                     # chunks per output DMA group
NGRP = NCHUNK // GRP        # 3
OUTG = GRP * CHUNK * MULT   # 7168 f32 per out group

_cache = {}
XDT = "f16"  # input/matmul operand dtype: "f32r" or "f16"
ODT = "f16"  # output wire dtype: "f32" or "f16" (host upcasts; rel-err ~3e-4)


def _build():
    import concourse.bacc as bacc
    import concourse.tile as tile
    from concourse import mybir

    f32 = mybir.dt.float32
    f32r = mybir.dt.float32r if XDT == "f32r" else mybir.dt.float16
    odt = mybir.dt.float32 if ODT == "f32" else mybir.dt.float16

    nc = bacc.Bacc("TRN2", target_bir_lowering=False, debug=False)
    x_d = nc.dram_tensor("x", [BPC, H, W, C], f32r, kind="ExternalInput")
    bands_d = nc.dram_tensor("bands", [KP, 12 * MB], f32r, kind="ExternalInput")
    brows_d = nc.dram_tensor("brows", [MULT, COLS], f32r, kind="ExternalInput")
    out_d = nc.dram_tensor("out", [BPC, H, W, C * MULT], odt, kind="ExternalOutput")

    with tile.TileContext(nc) as tc:
        with (
            tc.tile_pool(name="const", bufs=1) as const,
            tc.tile_pool(name="xp", bufs=3) as xp,
            tc.tile_pool(name="op", bufs=3) as op,
            tc.tile_pool(name="ps", bufs=4, space="PSUM") as ps,
        ):
            band_t = const.tile([KP, 12 * MB], f32r)
            # bands + first bias rows ride the scalar (ACT) HWDGE queue so they
            # don't serialize ahead of the x pieces on the sync queue; finely
            # laddered (first-needed bytes first) because the cold DMA path
            # only sustains ~50 GB/s for the first few us.  Emission of the
            # ht=0 half happens inside the first tile (after the bias sliver);
            # the ht=1 half is deferred into tile 2 (needed only from tile 3).

            ev = 0  # eviction round-robin DVE/ACT
            for b in range(BPC):
                for ht in range(2):
                    h0 = ht * M
                    hs = 0 if ht == 0 else 110
                    for wh in range(2):
                        w0 = wh * WH
                        ws = 0 if wh == 0 else 110
                        # jk tap offset in tile cols: col = flat + 96*(jk-1) for
                        # wh=0 (tile holds w 0..113), col = flat + 96*(jk+1) for
                        # wh=1 (tile holds w 110..223).  The single out-of-range
                        # (chunk, jk) at each image w-edge is clipped to N=416 —
                        # the dropped 96 columns are exactly the SAME-pad taps.
                        joff = -1 if wh == 0 else 1
                        jorder = (1, 2, 0) if wh == 0 else (1, 0, 2)
                        xt = xp.tile([KP, COLS], f32r)
                        # pieces so the first chunks' matmuls start sooner;
                        # the first tile ladders finely to launch the output
                        # stream as early as possible.  Halo rows (112:114) and
                        # bias rows (114:KP) are interleaved right after the
                        # first piece: every matmul contracts over them, so
                        # they must land before ANY chunk can run.
                        first = b == 0 and ht == 0 and wh == 0
                        last = b == BPC - 1 and ht == 1 and wh == 1
                        if b == 0 and ht == 0 and wh == 1:
                            nc.scalar.dma_start(
                                band_t[:, 6 * MB : 12 * MB], bands_d[:, 6 * MB : 12 * MB]
                            )
                        wsplit = (0, 8, 20, 45, 114) if first else (0, 57, 114)
                        for wa, wb_ in zip(wsplit, wsplit[1:]):
                            nc.sync.dma_start(
                                xt[0:112, wa * C : wb_ * C],
                                x_d[b, hs : hs + 112, ws + wa : ws + wb_, :],
                            )
                            nc.sync.dma_start(
                                xt[112:114, wa * C : wb_ * C],
                                x_d[b, hs + 112 : hs + 114, ws + wa : ws + wb_, :],
                            )
                            if wa == 0 and first:
                                w1 = wsplit[1] * C
                                nc.scalar.dma_start(
                                    xt[114:KP, 0:w1], brows_d[:, 0:w1]
                                )
                                nc.scalar.dma_start(
                                    band_t[:, 0 : 3 * MB], bands_d[:, 0 : 3 * MB]
                                )
                                nc.scalar.dma_start(
                                    band_t[:, 3 * MB : 6 * MB],
                                    bands_d[:, 3 * MB : 6 * MB],
                                )
                                nc.scalar.dma_start(
                                    xt[114:KP, w1:], brows_d[:, w1:]
                                )
                            elif wa == 0:
                                nc.sync.dma_start(xt[114:KP, :], brows_d[:, :])

                        od = out_d[b].rearrange("h w c -> h (w c)")
                        # taper: fine groups at the very start (first output DMA
                        # launches ASAP) and at the very end (last output DMA is
                        # small, shrinking the drain tail)
                        if first:
                            groups = (1, 2, 4, 7, 7)
                        elif last:
                            groups = (7, 7, 4, 2, 1)
                        else:
                            groups = (GRP,) * NGRP
                        ch = 0
                        for gi, gsz in enumerate(groups):
                            og = op.tile([M, GRP * CHUNK * MULT], odt, tag="og")
                            gbase = ch
                            for q in range(gsz):
                                n0 = ch * CHUNK
                                pt = ps.tile([MB, 2 * CHUNK], f32)
                                dst = og[:, q * 1024 : (q + 1) * 1024].rearrange(
                                    "p (n m) -> p n m", m=2
                                )
                                for m in range(MULT):
                                    for idx, jk in enumerate(jorder):
                                        bi = ht * 6 + m * 3 + jk
                                        c0 = n0 + 96 * (jk + joff)
                                        p0, p1 = 0, CHUNK
                                        if c0 < 0:
                                            p0, c0 = -c0, 0
                                        elif c0 + CHUNK > COLS:
                                            p1 = COLS - c0
                                        nc.tensor.matmul(
                                            pt[:, m * CHUNK + p0 : m * CHUNK + p1],
                                            band_t[:, bi * MB : (bi + 1) * MB],
                                            xt[0:KP, c0 : c0 + (p1 - p0)],
                                            start=(idx == 0),
                                            stop=(idx == 2),
                                        )
                                    # drain this m-half now (its accumulation
                                    # group just closed): PSUM returns ~0.6us
                                    # earlier per chunk, countering the drain-
                                    # lag phase erosion behind PE
                                    srcm = pt[0:M, m * CHUNK : (m + 1) * CHUNK]
                                    eng = (ev + (m if last and ch >= NCHUNK - 2 else 0)) % 2
                                    if eng == 0:
                                        nc.vector.tensor_copy(dst[:, :, m], srcm)
                                    else:
                                        nc.scalar.copy(dst[:, :, m], srcm)
                                ev += 1
                                ch += 1
                            cb = w0 * C * MULT + gbase * CHUNK * MULT
                            glen = gsz * CHUNK * MULT
                            if last and gi == len(groups) - 1:
                                # final group: split across both HWDGE queues so
                                # the closing transfer runs at 2x queue rate
                                half = glen // 2
                                nc.sync.dma_start(
                                    od[h0 : h0 + M, cb : cb + half], og[:, 0:half]
                                )
                                nc.scalar.dma_start(
                                    od[h0 : h0 + M, cb + half : cb + glen],
                                    og[:, half:glen],
                                )
                            elif last and gi == len(groups) - 2:
                                # penultimate group rides the idle sync queue so
                                # its transfer overlaps the final group's
                                nc.sync.dma_start(
                                    od[h0 : h0 + M, cb : cb + glen], og[:, 0:glen]
                                )
                            else:
                                # NOTE: steady-state group DMAs must stay on the
                                # scalar ring — issuing them from Sync serializes
                                # outputs ahead of input x pieces on one FIFO
                                # ring and starves PE at tile boundaries (+40us)
                                nc.scalar.dma_start(
                                    od[h0 : h0 + M, cb : cb + glen], og[:, 0:glen]
                                )
    nc.compile()
    return nc


def _host_consts(kern, bias):
    kk = np.asarray(kern, np.float32).reshape(3, 3, MULT)  # [dh, dw, m]
    bands = np.zeros((12, KP, MB), np.float32)  # cols 112..127 zero-padded (FWL)
    for ht in range(2):
        for m in range(MULT):
            for jk in range(3):
                band = bands[ht * 6 + m * 3 + jk]
                for i in range(3):
                    if ht == 0:
                        # tile row k holds x row h=k; out j needs rows j+i-1
                        ks = np.arange(M) + i - 1
                    else:
                        # tile row k holds x row h=110+k; out h=112+j reads
                        # h_in=111+j+i -> k=1+j+i (h_in=224 dropped: SAME pad)
                        ks = np.arange(M) + i + 1
                    js = np.arange(M)
                    sel = (ks >= 0) & (ks <= 113)
                    band[ks[sel], js[sel]] = kk[i, jk, m]
                if jk == 1:
                    band[114 + m, 0:M] = 1.0
    bands = np.ascontiguousarray(bands.transpose(1, 0, 2).reshape(KP, 12 * MB))
    brows = np.empty((MULT, COLS), np.float32)
    for m in range(MULT):
        brows[m] = np.tile(np.asarray(bias, np.float32)[m::MULT], WH + 2)
    return bands, brows


def kernel(**inputs):
    dt = np.float32 if XDT == "f32r" else np.float16
    x = np.ascontiguousarray(np.asarray(inputs["x"]).astype(dt))
    bands, brows = _host_consts(inputs["kernel"], inputs["bias"])
    bands = bands.astype(dt)
    brows = brows.astype(dt)

    if "nc" not in _cache:
        _cache["nc"] = _build()
    nc = _cache["nc"]

    from concourse.bass_utils import run_bass_kernel_spmd

    in_maps = [
        {"x": x[i * BPC : (i + 1) * BPC], "bands": bands, "brows": brows}
        for i in range(NCORES)
    ]
    res = run_bass_kernel_spmd(nc, in_maps, list(range(NCORES)))
    out = np.concatenate([res.results[i]["out"] for i in range(NCORES)], axis=0)
    return np.ascontiguousarray(out.astype(np.float32))



# revision 45
# speedup vs baseline: 1.1794x; 1.1794x over previous
"""Depthwise 3x3 conv (SAME, channel multiplier 2) on [16,224,224,96] f32,
data-parallel over batch across 8 TRN2 NeuronCores.

Per-core mapping (2 images/core): the conv along H is expressed as a banded
matmul on TensorE — stationary [116,112] band matrix whose 3 diagonals hold
the kernel column k[:, dw, m]; contract dim = 114 input rows (112 out rows +
halo) + 2 bias rows (all-ones coefficients fold the bias add into the PE).
The 3 W-shifts accumulate into PSUM via the moving operand's column offset
(dw*96 in the flattened (w,c) free dim).  Operands stream as f16 (matmul
cost is N moving columns @2.4GHz regardless of dtype/K; PE is the wall at
~217us/core: 24 passes over the (w,c) plane is a conserved quantity of any
banded formulation — M<=128 PSUM partitions x 3 taps x 2 mult).

PSUM [112, 512] per (chunk, m) pairs into [112,1024] tiles; DVE/ACT
interleave-copy them (out ch = 2c+m -> stride-2 write) into SBUF groups of 7
chunks.  The output is written to HBM as f16 and upcast on the host — the
2e-2 rel-err budget absorbs the ~3e-4 rounding and it halves the dominant
(write) HBM stream: 96 -> 58 MB/core total traffic.

Schedule trims: first tile ladders x/bias/band loads finely across both
HWDGE queues (cold DMA only sustains ~50GB/s for the first ~5us; first
matmul ~12.7us), last tile tapers output groups (7,7,4,2,1), splits the
final two chunks' drains across DVE+ACT and spreads the closing DMAs over
both queues.  Measured 247us/core (335us baseline): ~12.7us ramp + ~225us
PE-bound stream + ~8us tail.  NOTE the device has two clock states —
216ns/MM (2.4GHz) and 259ns/MM (~2.0GHz, thermal): compare runs only at
matching median MM gap.  Measured neutral-or-worse: FWL 128-col pad
(neutral — LDWEIGHTS already hidden), fp8 (fails 2e-2), every-chunk split
drains (+3us), interleaved stride-2 matmul PSUM dst (breaks accumulation),
engine direct-conv offload (engines are partition-locked; conv taps need
cross-partition reads).
"""

import sys

sys.path.insert(0, "/opt/trn_rl_repo")

import numpy as np

B, H, W, C = 16, 224, 224, 96
MULT = 2
NCORES = 8
BPC = B // NCORES  # images per core
M = 112            # output rows per h-tile
MB = 112           # band stationary columns (128-pad triggers FWL: measured neutral —
                   # LDWEIGHTS 131->98ns but already hidden under the 216ns/MM stream)
KP = 116           # contract partitions: 114 x rows + 2 bias rows
WH = 112           # w-half width
COLS = (WH + 2) * C         # 10944 x-tile cols (1-w halo each side)
CHUNK = 512
NCHUNK = WH * C // CHUNK    # 21
GRP = 7                     # chunks per output DMA group
NGRP = NCHUNK // GRP        # 3
OUTG = GRP * CHUNK * MULT   # 7168 f32 per out group

_cache = {}
XDT = "f16"  # input/matmul operand dtype: "f32r" or "f16"
ODT = "f16"  # output wire dtype: "f32" or "f16" (host upcasts; rel-err ~3e-4)


def _build():
    import concourse.bacc as bacc
    import concourse.tile as tile
    from concourse import mybir

    f32 = mybir.dt.float32
    f32r = mybir.dt.float32r if XDT == "f32r" else mybir.dt.float16
    odt = mybir.dt.float32 if ODT == "f32" else mybir.dt.float16

    nc = bacc.Bacc("TRN2", target_bir_lowering=False, debug=False)
    x_d = nc.dram_tensor("x", [BPC, H, W, C], f32r, kind="ExternalInput")
    bands_d = nc.dram_tensor("bands", [KP, 12 * MB], f32r, kind="ExternalInput")
    brows_d = nc.dram_tensor("brows", [MULT, COLS], f32r, kind="ExternalInput")
    out_d = nc.dram_tensor("out", [BPC, H, W, C * MULT], odt, kind="ExternalOutput")

    with tile.TileContext(nc) as tc:
        with (
            tc.tile_pool(name="const", bufs=1) as const,
            tc.tile_pool(name="xp", bufs=3) as xp,
            tc.tile_pool(name="op", bufs=3) as op,
            tc.tile_pool(name="ps", bufs=4, space="PSUM") as ps,
        ):
            band_t = const.tile([KP, 12 * MB], f32r)
            # bands + first bias rows ride the scalar (ACT) HWDGE queue so they
            # don't serialize ahead of the x pieces on the sync queue; finely
            # laddered (first-needed bytes first) because the cold DMA path
            # only sustains ~50 GB/s for the first few us.  Emission of the
            # ht=0 half happens inside the first tile (after the bias sliver);
            # the ht=1 half is deferred into tile 2 (needed only from tile 3).

            ev = 0  # eviction round-robin DVE/ACT
            for b in range(BPC):
                for ht in range(2):
                    h0 = ht * M
                    hs = 0 if ht == 0 else 110
                    for wh in range(2):
                        w0 = wh * WH
                        ws = 0 if wh == 0 else 110
                        # jk tap offset in tile cols: col = flat + 96*(jk-1) for
                        # wh=0 (tile holds w 0..113), col = flat + 96*(jk+1) for
                        # wh=1 (tile holds w 110..223).  The single out-of-range
                        # (chunk, jk) at each image w-edge is clipped to N=416 —
                        # the dropped 96 columns are exactly the SAME-pad taps.
                        joff = -1 if wh == 0 else 1
                        jorder = (1, 2, 0) if wh == 0 else (1, 0, 2)
                        xt = xp.tile([KP, COLS], f32r)
                        # pieces so the first chunks' matmuls start sooner;
                        # the first tile ladders finely to launch the output
                        # stream as early as possible.  Halo rows (112:114) and
                        # bias rows (114:KP) are interleaved right after the
                        # first piece: every matmul contracts over them, so
                        # they must land before ANY chunk can run.
                        first = b == 0 and ht == 0 and wh == 0
                        last = b == BPC - 1 and ht == 1 and wh == 1
                        if b == 0 and ht == 0 and wh == 1:
                            nc.scalar.dma_start(
                                band_t[:, 6 * MB : 12 * MB], bands_d[:, 6 * MB : 12 * MB]
                            )
                        wsplit = (0, 8, 20, 45, 114) if first else (0, 57, 114)
                        for wa, wb_ in zip(wsplit, wsplit[1:]):
                            nc.sync.dma_start(
                                xt[0:112, wa * C : wb_ * C],
                                x_d[b, hs : hs + 112, ws + wa : ws + wb_, :],
                            )
                            nc.sync.dma_start(
                                xt[112:114, wa * C : wb_ * C],
                                x_d[b, hs + 112 : hs + 114, ws + wa : ws + wb_, :],
                            )
                            if wa == 0 and first:
                                w1 = wsplit[1] * C
                                nc.scalar.dma_start(
                                    xt[114:KP, 0:w1], brows_d[:, 0:w1]
                                )
                                nc.scalar.dma_start(
                                    band_t[:, 0 : 3 * MB], bands_d[:, 0 : 3 * MB]
                                )
                                nc.scalar.dma_start(
                                    band_t[:, 3 * MB : 6 * MB],
                                    bands_d[:, 3 * MB : 6 * MB],
                                )
                                nc.scalar.dma_start(
                                    xt[114:KP, w1:], brows_d[:, w1:]
                                )
                            elif wa == 0:
                                nc.sync.dma_start(xt[114:KP, :], brows_d[:, :])

                        od = out_d[b].rearrange("h w c -> h (w c)")
                        # taper: fine groups at the very start (first output DMA
                        # launches ASAP) and at the very end (last output DMA is
                        # small, shrinking the drain tail)
                        if first:
                            groups = (1, 2, 4, 7, 7)
                        elif last:
                            groups = (7, 7, 4, 2, 1)
                        else:
                            groups = (GRP,) * NGRP
                        ch = 0
                        for gi, gsz in enumerate(groups):
                            og = op.tile([M, GRP * CHUNK * MULT], odt, tag="og")
                            gbase = ch
                            for q in range(gsz):
                                n0 = ch * CHUNK
                                pt = ps.tile([MB, 2 * CHUNK], f32)
                                for m in range(MULT):
                                    for idx, jk in enumerate(jorder):
                                        bi = ht * 6 + m * 3 + jk
                                        c0 = n0 + 96 * (jk + joff)
                                        p0, p1 = 0, CHUNK
                                        if c0 < 0:
                                            p0, c0 = -c0, 0
                                        elif c0 + CHUNK > COLS:
                                            p1 = COLS - c0
                                        nc.tensor.matmul(
                                            pt[:, m * CHUNK + p0 : m * CHUNK + p1],
                                            band_t[:, bi * MB : (bi + 1) * MB],
                                            xt[0:KP, c0 : c0 + (p1 - p0)],
                                            start=(idx == 0),
                                            stop=(idx == 2),
                                        )
                                # one whole-chunk interleave copy per chunk,
                                # engines alternating: half-chunk drains and
                                # every-chunk engine splits both measured WORSE
                                # (each gated drain op adds sem overhead to the
                                # chasing drain pipeline)
                                src = pt[0:M, :].rearrange("p (m n) -> p n m", m=2)
                                dst = og[:, q * 1024 : (q + 1) * 1024].rearrange(
                                    "p (n m) -> p n m", m=2
                                )
                                if last and ch >= NCHUNK - 2:
                                    # final two chunks are on the drain critical
                                    # path: split their copies across engines
                                    nc.vector.tensor_copy(dst[:, :, 0], src[:, :, 0])
                                    nc.scalar.copy(dst[:, :, 1], src[:, :, 1])
                                elif ev % 2 == 0:
                                    nc.vector.tensor_copy(dst, src)
                                else:
                                    nc.scalar.copy(dst, src)
                                ev += 1
                                ch += 1
                            cb = w0 * C * MULT + gbase * CHUNK * MULT
                            glen = gsz * CHUNK * MULT
                            if last and gi == len(groups) - 1:
                                # final group: split across both HWDGE queues so
                                # the closing transfer runs at 2x queue rate
                                half = glen // 2
                                nc.sync.dma_start(
                                    od[h0 : h0 + M, cb : cb + half], og[:, 0:half]
                                )
                                nc.scalar.dma_start(
                                    od[h0 : h0 + M, cb + half : cb + glen],
                                    og[:, half:glen],
                                )
                            elif last and gi == len(groups) - 2:
                                # penultimate group rides the idle sync queue so
                                # its transfer overlaps the final group's
                                nc.sync.dma_start(
                                    od[h0 : h0 + M, cb : cb + glen], og[:, 0:glen]
                                )
                            else:
                                # NOTE: steady-state group DMAs must stay on the
                                # scalar ring — issuing them from Sync serializes
                                # outputs ahead of input x pieces on one FIFO
                                # ring and starves PE at tile boundaries (+40us)
                                nc.scalar.dma_start(
                                    od[h0 : h0 + M, cb : cb + glen], og[:, 0:glen]
                                )
    nc.compile()
    return nc


def _host_consts(kern, bias):
    kk = np.asarray(kern, np.float32).reshape(3, 3, MULT)  # [dh, dw, m]
    bands = np.zeros((12, KP, MB), np.float32)  # cols 112..127 zero-padded (FWL)
    for ht in range(2):
        for m in range(MULT):
            for jk in range(3):
                band = bands[ht * 6 + m * 3 + jk]
                for i in range(3):
                    if ht == 0:
                        # tile row k holds x row h=k; out j needs rows j+i-1
                        ks = np.arange(M) + i - 1
                    else:
                        # tile row k holds x row h=110+k; out h=112+j reads
                        # h_in=111+j+i -> k=1+j+i (h_in=224 dropped: SAME pad)
                        ks = np.arange(M) + i + 1
                    js = np.arange(M)
                    sel = (ks >= 0) & (ks <= 113)
                    band[ks[sel], js[sel]] = kk[i, jk, m]
                if jk == 1:
                    band[114 + m, 0:M] = 1.0
    bands = np.ascontiguousarray(bands.transpose(1, 0, 2).reshape(KP, 12 * MB))
    brows = np.empty((MULT, COLS), np.float32)
    for m in range(MULT):
        brows[m] = np.tile(np.asarray(bias, np.float32)[m::MULT], WH + 2)
    return bands, brows


def kernel(**inputs):
    dt = np.float32 if XDT == "f32r" else np.float16
    x = np.ascontiguousarray(np.asarray(inputs["x"]).astype(dt))
    bands, brows = _host_consts(inputs["kernel"], inputs["bias"])
    bands = bands.astype(dt)
    brows = brows.astype(dt)

    if "nc" not in _cache:
        _cache["nc"] = _build()
    nc = _cache["nc"]

    from concourse.bass_utils import run_bass_kernel_spmd

    in_maps = [
        {"x": x[i * BPC : (i + 1) * BPC], "bands": bands, "brows": brows}
        for i in range(NCORES)
    ]
    res = run_bass_kernel_spmd(nc, in_maps, list(range(NCORES)))
    out = np.concatenate([res.results[i]["out"] for i in range(NCORES)], axis=0)
    return np.ascontiguousarray(out.astype(np.float32))

